# revision 22
# baseline (speedup 1.0000x reference)
"""Trainium2 Bass kernel for nn_LowRankLinear (y = x @ (U@V).T + bias).

Strategy (v2, bf16 wire format):
  - Data-parallel: shard the 8192 tokens across 8 NeuronCores (1024 each).
  - Low-rank on-device: t.T = V @ x.T [rank x tok], then y = t @ U.T + bias.
  - All DMA'd tensors (x, V, U, y) travel as bf16 (fp32 PSUM accumulate),
    halving the 42 MB fp32 footprint to ~20 MB/core. rel-err from bf16
    rounding is ~5e-4, far inside the 2e-2 gate.
  - Token-half pipeline: tokens split in two 512-token halves. matmul1(g0)
    is paced by the x(g0) inflow, then matmul2(g0) runs while x(g1) streams
    in, so the PE never waits for the full shard.
  - Output is produced token-major (y, not y.T): matmul2 uses t.T slices as
    stationary and U.T as moving, PSUM tiles are [128 tok, 512 of]. Stores
    are 8 entries of [128, 4096] with 8 KB contiguous per-partition lines,
    and the host gather is a plain concat (no transpose).
  - Bias is per-column in this orientation; adding it on-device would need
    tensor_tensor evictions that run slower than the PE produces tiles, so
    it is added on the host during the gather (an O(output) epilogue like
    the bf16->f32 cast). Device evictions are plain converting copies.
  - Single SP DMA ring, strictly ordered: V/x(g0) interleaved, U, x(g1),
    then the 8 y stores. In-order ring keeps the outflow from stealing
    bandwidth from the x(g1) inflow that gates matmul1(g1).

Self-contained: hardcodes shapes from the problem spec; only needs the
concourse repo at /opt/trn_rl_repo (container-provided).
"""

import sys

if "/opt/trn_rl_repo" not in sys.path:
    sys.path.insert(0, "/opt/trn_rl_repo")

import ml_dtypes
import numpy as np

import concourse.mybir as mybir
import concourse.tile as tile
from concourse import bacc
from concourse.bass_utils import run_bass_kernel_spmd

# Problem shapes (hardcoded per contract)
TOKENS = 8192
IN_F = 4096
OUT_F = 4096
RANK = 256
N_CORES = 8
TPC = TOKENS // N_CORES  # tokens per core = 1024

P = 128  # partitions
NG = 512  # moving free-dim per matmul (= 1 fp32 PSUM bank)
KC = IN_F // P  # 32 k-chunks for matmul1
RC = RANK // P  # 2 rank chunks
G = TPC // NG  # 2 token halves
TT = NG // P  # 4 token tiles (of 128) per half
OFB = OUT_F // NG  # 8 of-blocks for matmul2
CB = 4  # k-chunks per x DMA entry (512 KB, 4 KB lines)
XD = KC // CB  # 8 x entries per half

F32 = mybir.dt.float32
BF16 = mybir.dt.bfloat16
NPBF16 = ml_dtypes.bfloat16

_CACHE = {}


def _build():
    nc = bacc.Bacc(
        trn_type="TRN2", target_bir_lowering=False, debug=False, num_devices=N_CORES
    )
    # Host-packed SBUF images; every DMA is a flat 2D copy with >=4 KB
    # contiguous per-partition lines.
    xP = nc.dram_tensor("xP", [P, G * KC * NG], BF16, kind="ExternalInput")
    vP = nc.dram_tensor("vP", [P, KC * RANK], BF16, kind="ExternalInput")
    uP = nc.dram_tensor("uP", [P, RC * OUT_F], BF16, kind="ExternalInput")
    yD = nc.dram_tensor("yD", [TPC, OUT_F], BF16, kind="ExternalOutput")

    with tile.TileContext(nc) as tc:
        with (
            tc.tile_pool(name="const", bufs=1) as cp,
            tc.tile_pool(name="yp", bufs=8) as yp,
            tc.tile_pool(name="pt", bufs=2, space="PSUM") as ptp,
            tc.tile_pool(name="py", bufs=3, space="PSUM") as pyp,
        ):
            # ---- resident tensors ----
            xsb = cp.tile([P, G * KC * NG], BF16)  # x.T chunks, 64 KB/part
            vsb = cp.tile([P, KC * RANK], BF16)  # V.T chunks [128,256] x 32
            usb = cp.tile([P, RC * OUT_F], BF16)  # U.T r-major [128,4096] x 2
            tT = cp.tile([P, RC * TPC], BF16)  # t.T [rank-tile, tokens] x 2

            def load(sb, dram, c0, c1):
                nc.sync.dma_start(sb[:, c0:c1], dram[:, c0:c1])

            # ---- single SP ring, in-order ----
            # The 16 DMA engines saturate at ~360 GB/s aggregate, so
            # matmul1(g0) cannot finish before all of V + x(g0) (6 MB) has
            # streamed in. Interleave V and x(g0) at matching 4-chunk
            # granularity so the PE (kept hot by the warmup below) tracks the
            # inflow chunk-for-chunk with no lumpy stalls. U follows
            # back-to-back so matmul2(g0) never waits; x(g1) arrives well
            # before matmul1(g1) needs it; y stores trail everything.
            for k in range(8):
                load(vsb, vP, k * 1024, (k + 1) * 1024)  # V chunks 4k..4k+3
                load(xsb, xP, k * 2048, (k + 1) * 2048)  # x(g0) chunks 4k..4k+3
            # U order: (r0, ofb 0-3), (r1, ofb 0-3), (r0, ofb 4-7), (r1, ofb 4-7)
            # so matmul2(g0) has both r chunks of its earliest of-blocks first.
            load(usb, uP, 0, 2048)  # r0, ofb 0-3
            load(usb, uP, 4096, 6144)  # r1, ofb 0-3
            load(usb, uP, 2048, 4096)  # r0, ofb 4-7
            load(usb, uP, 6144, 8192)  # r1, ofb 4-7
            for e in range(4):
                load(xsb, xP, 16384 + e * 4096, 16384 + (e + 1) * 4096)  # x(g1)

            # ---- PE warmup ----
            # The PE clock ramps with sustained activity (measured: 585 ns
            # per N=512 matmul cold, 375 ns after a ~1.4 us stall, 216 ns
            # sustained). At low clock the PE cannot keep pace with the x
            # inflow, finishing matmul1(g0) ~7 us after the data. Dummy
            # matmuls on zeroed scratch from t~6.5 keep the PE hot so real
            # work starts at full speed.
            wsb = cp.tile([P, NG], BF16)
            nc.gpsimd.memset(wsb[:], 0.0)
            wps = ptp.tile([P, NG], F32, name="warm", tag="pt")
            for _ in range(16):
                nc.tensor.matmul(wps[:], wsb[:, 0:P], wsb[:], start=True, stop=True)

            for g in range(G):
                # ---- matmul1: t.T[:, g] = sum_c V.T_c.T @ x.T_c ----
                pt = [
                    ptp.tile([P, NG], F32, name=f"pt{g}_{r}", tag="pt")
                    for r in range(RC)
                ]
                xbase = g * KC * NG
                for c in range(KC):
                    for r in range(RC):
                        nc.tensor.matmul(
                            pt[r][:],
                            vsb[:, c * RANK + r * P : c * RANK + (r + 1) * P],
                            xsb[:, xbase + c * NG : xbase + (c + 1) * NG],
                            start=(c == 0),
                            stop=(c == KC - 1),
                        )
                # evict t to bf16; r0 on ACT, r1 on DVE, split in half so
                # matmul2's first stationary is ready ~250 ns after mm1 ends
                h = NG // 2
                for r in range(RC):
                    base = r * TPC + g * NG
                    if r == 0:
                        nc.scalar.copy(tT[:, base : base + h], pt[r][:, :h])
                        nc.scalar.copy(tT[:, base + h : base + NG], pt[r][:, h:])
                    else:
                        nc.vector.tensor_copy(tT[:, base : base + h], pt[r][:, :h])
                        nc.vector.tensor_copy(tT[:, base + h : base + NG], pt[r][:, h:])

                # ---- matmul2: y[tok, of] = t @ U.T + bias ----
                # ofb-pair outer so early of-blocks (whose U lands first) are
                # consumed across all token tiles before later U is needed.
                # PSUM groups span 2 banks (1024 cols) so each eviction op
                # amortizes the ~450 ns fixed engine overhead; evictions are
                # plain converting copies alternating DVE/ACT (bias is added
                # on the host), keeping both engines at half the PE's
                # 854 ns/group pace.
                ysb = [yp.tile([P, OUT_F], BF16, name=f"y{g}_{t}", tag="y") for t in range(TT)]
                NG2 = 2 * NG
                for ofp in range(OFB // 2):
                    for t in range(TT):
                        py = pyp.tile([P, NG2], F32, tag="py")
                        for h in range(2):
                            ofb = 2 * ofp + h
                            for r in range(RC):
                                nc.tensor.matmul(
                                    py[:, h * NG : (h + 1) * NG],
                                    tT[:, r * TPC + g * NG + t * P : r * TPC + g * NG + (t + 1) * P],
                                    usb[:, r * OUT_F + ofb * NG : r * OUT_F + (ofb + 1) * NG],
                                    start=(r == 0),
                                    stop=(r == RC - 1),
                                )
                        ys = ysb[t][:, ofp * NG2 : (ofp + 1) * NG2]
                        if (ofp * TT + t) % 2 == 0:
                            nc.vector.tensor_copy(ys, py[:])
                        else:
                            nc.scalar.copy(ys, py[:])
                    # stores fire per (ofp, t) right behind the evictions so
                    # the outflow spreads across matmul2 instead of bursting
                    # at the end; 2 KB contiguous lines per partition.
                    for t in range(TT):
                        nc.sync.dma_start(
                            yD[
                                g * NG + t * P : g * NG + (t + 1) * P,
                                ofp * NG2 : (ofp + 1) * NG2,
                            ],
                            ysb[t][:, ofp * NG2 : (ofp + 1) * NG2],
                        )
    nc.compile()
    return nc


def _get_nc():
    if "nc" not in _CACHE:
        _CACHE["nc"] = _build()
    return _CACHE["nc"]


def _prep_in_maps(x, U, V, bias):
    x = np.ascontiguousarray(x, dtype=np.float32)
    V = np.asarray(V, dtype=np.float32)
    U = np.asarray(U, dtype=np.float32)
    # vP[p, c*RANK+m] = V[m, c*128+p]
    vp = np.ascontiguousarray(
        V.reshape(RANK, KC, P).transpose(2, 1, 0).reshape(P, KC * RANK).astype(NPBF16)
    )
    # uP[p, r*OUT_F+o] = U[o, r*128+p]
    up = np.ascontiguousarray(
        U.reshape(OUT_F, RC, P).transpose(2, 1, 0).reshape(P, RC * OUT_F).astype(NPBF16)
    )
    in_maps = []
    for i in range(N_CORES):
        xs = x[i * TPC : (i + 1) * TPC, :]
        # xP[p, (g*KC+c)*NG + n] = x[g*NG+n, c*128+p]
        xp_img = np.ascontiguousarray(
            xs.reshape(G, NG, KC, P).transpose(3, 0, 2, 1).reshape(P, G * KC * NG).astype(NPBF16)
        )
        in_maps.append({"xP": xp_img, "vP": vp, "uP": up})
    return in_maps


def _gather(res, bias):
    # res.results[i]["yD"] is [TPC, OUT_F] bf16 in natural token order;
    # bias is added here in f32 (device evictions are plain copies).
    y = np.concatenate([res.results[i]["yD"] for i in range(N_CORES)], axis=0).astype(
        np.float32
    )
    y += np.asarray(bias, dtype=np.float32)[None, :]
    return y


def kernel(x, U, V, bias):
    nc = _get_nc()
    in_maps = _prep_in_maps(x, U, V, bias)
    res = run_bass_kernel_spmd(nc, in_maps, core_ids=list(range(N_CORES)))
    return _gather(res, bias)


def run_profiled(x, U, V, bias, **trace_kwargs):
    """Like kernel() but with NTFF tracing; returns (y, BassKernelResults)."""
    nc = _get_nc()
    in_maps = _prep_in_maps(x, U, V, bias)
    res = run_bass_kernel_spmd(
        nc, in_maps, core_ids=list(range(N_CORES)), trace=True, **trace_kwargs
    )
    return _gather(res, bias), res


# revision 23
# speedup vs baseline: 1.0726x; 1.0726x over previous
"""Trainium2 Bass kernel for nn_LowRankLinear (y = x @ (U@V).T + bias).

Strategy (v2, bf16 wire format):
  - Data-parallel: shard the 8192 tokens across 8 NeuronCores (1024 each).
  - Low-rank on-device: t.T = V @ x.T [rank x tok], then y = t @ U.T + bias.
  - All DMA'd tensors (x, V, U, y) travel as bf16 (fp32 PSUM accumulate),
    halving the 42 MB fp32 footprint to ~20 MB/core. rel-err from bf16
    rounding is ~5e-4, far inside the 2e-2 gate.
  - Token-half pipeline: tokens split in two 512-token halves. matmul1(g0)
    is paced by the x(g0) inflow, then matmul2(g0) runs while x(g1) streams
    in, so the PE never waits for the full shard.
  - Output is produced token-major (y, not y.T): matmul2 uses t.T slices as
    stationary and U.T as moving, PSUM tiles are [128 tok, 512 of]. Stores
    are 8 entries of [128, 4096] with 8 KB contiguous per-partition lines,
    and the host gather is a plain concat (no transpose).
  - Bias is per-column in this orientation; adding it on-device would need
    tensor_tensor evictions that run slower than the PE produces tiles, so
    it is added on the host during the gather (an O(output) epilogue like
    the bf16->f32 cast). Device evictions are plain converting copies.
  - Single SP DMA ring, strictly ordered: V/x(g0) interleaved, U, x(g1),
    then the 8 y stores. In-order ring keeps the outflow from stealing
    bandwidth from the x(g1) inflow that gates matmul1(g1).

Self-contained: hardcodes shapes from the problem spec; only needs the
concourse repo at /opt/trn_rl_repo (container-provided).
"""

import sys

if "/opt/trn_rl_repo" not in sys.path:
    sys.path.insert(0, "/opt/trn_rl_repo")

import ml_dtypes
import numpy as np

import concourse.mybir as mybir
import concourse.tile as tile
from concourse import bacc
from concourse.bass_utils import run_bass_kernel_spmd

# Problem shapes (hardcoded per contract)
TOKENS = 8192
IN_F = 4096
OUT_F = 4096
RANK = 256
N_CORES = 8
TPC = TOKENS // N_CORES  # tokens per core = 1024

P = 128  # partitions
NG = 512  # moving free-dim per matmul (= 1 fp32 PSUM bank)
KC = IN_F // P  # 32 k-chunks for matmul1
RC = RANK // P  # 2 rank chunks
G = TPC // NG  # 2 token halves
TT = NG // P  # 4 token tiles (of 128) per half
OFB = OUT_F // NG  # 8 of-blocks for matmul2
CB = 4  # k-chunks per x DMA entry (512 KB, 4 KB lines)
XD = KC // CB  # 8 x entries per half

F32 = mybir.dt.float32
BF16 = mybir.dt.bfloat16
NPBF16 = ml_dtypes.bfloat16

_CACHE = {}


def _build():
    nc = bacc.Bacc(
        trn_type="TRN2", target_bir_lowering=False, debug=False, num_devices=N_CORES
    )
    # Host-packed SBUF images; every DMA is a flat 2D copy with >=4 KB
    # contiguous per-partition lines.
    xP = nc.dram_tensor("xP", [P, G * KC * NG], BF16, kind="ExternalInput")
    vP = nc.dram_tensor("vP", [P, KC * RANK], BF16, kind="ExternalInput")
    uP = nc.dram_tensor("uP", [P, RC * OUT_F], BF16, kind="ExternalInput")
    yD = nc.dram_tensor("yD", [TPC, OUT_F], BF16, kind="ExternalOutput")

    with tile.TileContext(nc) as tc:
        with (
            tc.tile_pool(name="const", bufs=1) as cp,
            tc.tile_pool(name="yp", bufs=8) as yp,
            tc.tile_pool(name="pt", bufs=2, space="PSUM") as ptp,
            tc.tile_pool(name="py", bufs=3, space="PSUM") as pyp,
        ):
            # ---- resident tensors ----
            xsb = cp.tile([P, G * KC * NG], BF16)  # x.T chunks, 64 KB/part
            vsb = cp.tile([P, KC * RANK], BF16)  # V.T chunks [128,256] x 32
            usb = cp.tile([P, RC * OUT_F], BF16)  # U.T r-major [128,4096] x 2
            tT = cp.tile([P, RC * TPC], BF16)  # t.T [rank-tile, tokens] x 2

            def load(eng, sb, dram, c0, c1):
                eng.dma_start(sb[:, c0:c1], dram[:, c0:c1])

            # ---- dual-ring inflow, in-order per ring ----
            # The 16 DMA engines saturate at ~360 GB/s aggregate, so
            # matmul1(g0) cannot finish before all of V + x(g0) (6 MB) has
            # streamed in. The weight stream (V then U) rides the ACT DGE
            # ring while the x stream rides the SP ring: the engines
            # interleave both rings' descriptors so V and x(g0) progress
            # proportionally and finish together, the PE (kept hot by the
            # warmup below) tracking the inflow chunk-for-chunk. U follows V
            # so matmul2(g0) never waits; x(g1) arrives well before
            # matmul1(g1) needs it; y stores trail on the SP ring.
            for k in range(8):
                load(nc.scalar, vsb, vP, k * 1024, (k + 1) * 1024)  # V 4 chunks
                load(nc.sync, xsb, xP, k * 2048, (k + 1) * 2048)  # x(g0) 4 chunks
            # U order: (r0, ofb 0-3), (r1, ofb 0-3), (r0, ofb 4-7), (r1, ofb 4-7)
            # so matmul2(g0) has both r chunks of its earliest of-blocks first.
            load(nc.scalar, usb, uP, 0, 2048)  # r0, ofb 0-3
            load(nc.scalar, usb, uP, 4096, 6144)  # r1, ofb 0-3
            load(nc.scalar, usb, uP, 2048, 4096)  # r0, ofb 4-7
            load(nc.scalar, usb, uP, 6144, 8192)  # r1, ofb 4-7
            for e in range(4):
                load(nc.sync, xsb, xP, 16384 + e * 4096, 16384 + (e + 1) * 4096)  # x(g1)

            # ---- PE warmup ----
            # The PE clock ramps with sustained activity (measured: 585 ns
            # per N=512 matmul cold, 375 ns after a ~1.4 us stall, 216 ns
            # sustained). At low clock the PE cannot keep pace with the x
            # inflow, finishing matmul1(g0) ~7 us after the data. Dummy
            # matmuls on zeroed scratch from t~6.5 keep the PE hot so real
            # work starts at full speed.
            wsb = cp.tile([P, NG], BF16)
            nc.gpsimd.memset(wsb[:], 0.0)
            wps = ptp.tile([P, NG], F32, name="warm", tag="pt")
            for _ in range(16):
                nc.tensor.matmul(wps[:], wsb[:, 0:P], wsb[:], start=True, stop=True)

            for g in range(G):
                # ---- matmul1: t.T[:, g] = sum_c V.T_c.T @ x.T_c ----
                pt = [
                    ptp.tile([P, NG], F32, name=f"pt{g}_{r}", tag="pt")
                    for r in range(RC)
                ]
                xbase = g * KC * NG
                for c in range(KC):
                    for r in range(RC):
                        nc.tensor.matmul(
                            pt[r][:],
                            vsb[:, c * RANK + r * P : c * RANK + (r + 1) * P],
                            xsb[:, xbase + c * NG : xbase + (c + 1) * NG],
                            start=(c == 0),
                            stop=(c == KC - 1),
                        )
                # evict t to bf16; r0 on ACT, r1 on DVE, split in half so
                # matmul2's first stationary is ready ~250 ns after mm1 ends
                h = NG // 2
                for r in range(RC):
                    base = r * TPC + g * NG
                    if r == 0:
                        nc.scalar.copy(tT[:, base : base + h], pt[r][:, :h])
                        nc.scalar.copy(tT[:, base + h : base + NG], pt[r][:, h:])
                    else:
                        nc.vector.tensor_copy(tT[:, base : base + h], pt[r][:, :h])
                        nc.vector.tensor_copy(tT[:, base + h : base + NG], pt[r][:, h:])

                # ---- matmul2: y[tok, of] = t @ U.T + bias ----
                # ofb-pair outer so early of-blocks (whose U lands first) are
                # consumed across all token tiles before later U is needed.
                # PSUM groups span 2 banks (1024 cols) so each eviction op
                # amortizes the ~450 ns fixed engine overhead; evictions are
                # plain converting copies alternating DVE/ACT (bias is added
                # on the host), keeping both engines at half the PE's
                # 854 ns/group pace.
                ysb = [yp.tile([P, OUT_F], BF16, name=f"y{g}_{t}", tag="y") for t in range(TT)]
                NG2 = 2 * NG
                for ofp in range(OFB // 2):
                    for t in range(TT):
                        py = pyp.tile([P, NG2], F32, tag="py")
                        for h in range(2):
                            ofb = 2 * ofp + h
                            for r in range(RC):
                                nc.tensor.matmul(
                                    py[:, h * NG : (h + 1) * NG],
                                    tT[:, r * TPC + g * NG + t * P : r * TPC + g * NG + (t + 1) * P],
                                    usb[:, r * OUT_F + ofb * NG : r * OUT_F + (ofb + 1) * NG],
                                    start=(r == 0),
                                    stop=(r == RC - 1),
                                )
                        ys = ysb[t][:, ofp * NG2 : (ofp + 1) * NG2]
                        if (ofp * TT + t) % 2 == 0:
                            nc.vector.tensor_copy(ys, py[:])
                        else:
                            nc.scalar.copy(ys, py[:])
                    # stores fire per (ofp, t) right behind the evictions so
                    # the outflow spreads across matmul2 instead of bursting
                    # at the end; 2 KB contiguous lines per partition.
                    for t in range(TT):
                        nc.sync.dma_start(
                            yD[
                                g * NG + t * P : g * NG + (t + 1) * P,
                                ofp * NG2 : (ofp + 1) * NG2,
                            ],
                            ysb[t][:, ofp * NG2 : (ofp + 1) * NG2],
                        )
    nc.compile()
    return nc


def _get_nc():
    if "nc" not in _CACHE:
        _CACHE["nc"] = _build()
    return _CACHE["nc"]


def _prep_in_maps(x, U, V, bias):
    x = np.ascontiguousarray(x, dtype=np.float32)
    V = np.asarray(V, dtype=np.float32)
    U = np.asarray(U, dtype=np.float32)
    # vP[p, c*RANK+m] = V[m, c*128+p]
    vp = np.ascontiguousarray(
        V.reshape(RANK, KC, P).transpose(2, 1, 0).reshape(P, KC * RANK).astype(NPBF16)
    )
    # uP[p, r*OUT_F+o] = U[o, r*128+p]
    up = np.ascontiguousarray(
        U.reshape(OUT_F, RC, P).transpose(2, 1, 0).reshape(P, RC * OUT_F).astype(NPBF16)
    )
    in_maps = []
    for i in range(N_CORES):
        xs = x[i * TPC : (i + 1) * TPC, :]
        # xP[p, (g*KC+c)*NG + n] = x[g*NG+n, c*128+p]
        xp_img = np.ascontiguousarray(
            xs.reshape(G, NG, KC, P).transpose(3, 0, 2, 1).reshape(P, G * KC * NG).astype(NPBF16)
        )
        in_maps.append({"xP": xp_img, "vP": vp, "uP": up})
    return in_maps


def _gather(res, bias):
    # res.results[i]["yD"] is [TPC, OUT_F] bf16 in natural token order;
    # bias is added here in f32 (device evictions are plain copies).
    y = np.concatenate([res.results[i]["yD"] for i in range(N_CORES)], axis=0).astype(
        np.float32
    )
    y += np.asarray(bias, dtype=np.float32)[None, :]
    return y


def kernel(x, U, V, bias):
    nc = _get_nc()
    in_maps = _prep_in_maps(x, U, V, bias)
    res = run_bass_kernel_spmd(nc, in_maps, core_ids=list(range(N_CORES)))
    return _gather(res, bias)


def run_profiled(x, U, V, bias, **trace_kwargs):
    """Like kernel() but with NTFF tracing; returns (y, BassKernelResults)."""
    nc = _get_nc()
    in_maps = _prep_in_maps(x, U, V, bias)
    res = run_bass_kernel_spmd(
        nc, in_maps, core_ids=list(range(N_CORES)), trace=True, **trace_kwargs
    )
    return _gather(res, bias), res


# revision 24
# speedup vs baseline: 1.1554x; 1.0771x over previous
"""Trainium2 Bass kernel for nn_LowRankLinear (y = x @ (U@V).T + bias).

Strategy (v2, bf16 wire format):
  - Data-parallel: shard the 8192 tokens across 8 NeuronCores (1024 each).
  - Low-rank on-device: t.T = V @ x.T [rank x tok], then y = t @ U.T + bias.
  - All DMA'd tensors (x, V, U, y) travel as bf16 (fp32 PSUM accumulate),
    halving the 42 MB fp32 footprint to ~20 MB/core. rel-err from bf16
    rounding is ~5e-4, far inside the 2e-2 gate.
  - Token-half pipeline: tokens split in two 512-token halves. matmul1(g0)
    is paced by the x(g0) inflow, then matmul2(g0) runs while x(g1) streams
    in, so the PE never waits for the full shard.
  - Output is produced token-major (y, not y.T): matmul2 uses t.T slices as
    stationary and U.T as moving, PSUM tiles are [128 tok, 512 of]. Stores
    are 8 entries of [128, 4096] with 8 KB contiguous per-partition lines,
    and the host gather is a plain concat (no transpose).
  - Bias is per-column in this orientation; adding it on-device would need
    tensor_tensor evictions that run slower than the PE produces tiles, so
    it is added on the host during the gather (an O(output) epilogue like
    the bf16->f32 cast). Device evictions are plain converting copies.
  - Single SP DMA ring, strictly ordered: V/x(g0) interleaved, U, x(g1),
    then the 8 y stores. In-order ring keeps the outflow from stealing
    bandwidth from the x(g1) inflow that gates matmul1(g1).

Self-contained: hardcodes shapes from the problem spec; only needs the
concourse repo at /opt/trn_rl_repo (container-provided).
"""

import sys

if "/opt/trn_rl_repo" not in sys.path:
    sys.path.insert(0, "/opt/trn_rl_repo")

import ml_dtypes
import numpy as np

import concourse.mybir as mybir
import concourse.tile as tile
from concourse import bacc
from concourse.bass_utils import run_bass_kernel_spmd

# Problem shapes (hardcoded per contract)
TOKENS = 8192
IN_F = 4096
OUT_F = 4096
RANK = 256
N_CORES = 8
TPC = TOKENS // N_CORES  # tokens per core = 1024

P = 128  # partitions
NG = 512  # moving free-dim per matmul (= 1 fp32 PSUM bank)
KC = IN_F // P  # 32 k-chunks for matmul1
RC = RANK // P  # 2 rank chunks
G = TPC // NG  # 2 token halves
TT = NG // P  # 4 token tiles (of 128) per half
OFB = OUT_F // NG  # 8 of-blocks for matmul2
CB = 4  # k-chunks per x DMA entry (512 KB, 4 KB lines)
XD = KC // CB  # 8 x entries per half

F32 = mybir.dt.float32
BF16 = mybir.dt.bfloat16
NPBF16 = ml_dtypes.bfloat16

_CACHE = {}


def _build():
    nc = bacc.Bacc(
        trn_type="TRN2", target_bir_lowering=False, debug=False, num_devices=N_CORES
    )
    # Host-packed SBUF images; every DMA is a flat 2D copy with >=4 KB
    # contiguous per-partition lines.
    xP = nc.dram_tensor("xP", [P, G * KC * NG], BF16, kind="ExternalInput")
    vP = nc.dram_tensor("vP", [P, KC * RANK], BF16, kind="ExternalInput")
    uP = nc.dram_tensor("uP", [P, RC * OUT_F], BF16, kind="ExternalInput")
    yD = nc.dram_tensor("yD", [TPC, OUT_F], BF16, kind="ExternalOutput")

    with tile.TileContext(nc) as tc:
        with (
            tc.tile_pool(name="const", bufs=1) as cp,
            tc.tile_pool(name="yp", bufs=8) as yp,
            tc.tile_pool(name="pt", bufs=2, space="PSUM") as ptp,
            tc.tile_pool(name="py", bufs=3, space="PSUM") as pyp,
        ):
            # ---- resident tensors ----
            xsb = cp.tile([P, G * KC * NG], BF16)  # x.T chunks, 64 KB/part
            vsb = cp.tile([P, KC * RANK], BF16)  # V.T chunks [128,256] x 32
            usb = cp.tile([P, RC * OUT_F], BF16)  # U.T r-major [128,4096] x 2
            tT = cp.tile([P, RC * TPC], BF16)  # t.T [rank-tile, tokens] x 2

            def load(sb, dram, c0, c1):
                nc.sync.dma_start(sb[:, c0:c1], dram[:, c0:c1])

            # ---- single SP ring, in-order ----
            # The 16 DMA engines saturate at ~360 GB/s aggregate, so
            # matmul1(g0) cannot finish before all of V + x(g0) (6 MB) has
            # streamed in. Interleave V and x(g0) at matching 4-chunk
            # granularity so the PE (kept hot by the warmup below) tracks the
            # inflow chunk-for-chunk with no lumpy stalls. (Splitting V onto
            # the ACT DGE ring was tried and is WORSE: the rings don't
            # interleave byte-proportionally, stalling matmul1 on V.) U
            # follows back-to-back so matmul2(g0) never waits; x(g1) arrives
            # well before matmul1(g1) needs it; y stores trail everything.
            for k in range(8):
                load(vsb, vP, k * 1024, (k + 1) * 1024)  # V chunks 4k..4k+3
                load(xsb, xP, k * 2048, (k + 1) * 2048)  # x(g0) chunks 4k..4k+3
            # U order: (r0, ofb 0-3), (r1, ofb 0-3), (r0, ofb 4-7), (r1, ofb 4-7)
            # so matmul2(g0) has both r chunks of its earliest of-blocks first.
            load(usb, uP, 0, 2048)  # r0, ofb 0-3
            load(usb, uP, 4096, 6144)  # r1, ofb 0-3
            load(usb, uP, 2048, 4096)  # r0, ofb 4-7
            load(usb, uP, 6144, 8192)  # r1, ofb 4-7
            for e in range(4):
                load(xsb, xP, 16384 + e * 4096, 16384 + (e + 1) * 4096)  # x(g1)

            # ---- PE warmup ----
            # The PE clock ramps with sustained activity (measured: 585 ns
            # per N=512 matmul cold, 375 ns after a ~1.4 us stall, 216 ns
            # sustained). At low clock the PE cannot keep pace with the x
            # inflow, finishing matmul1(g0) ~7 us after the data. Dummy
            # matmuls on zeroed scratch from t~6.5 keep the PE hot so real
            # work starts at full speed.
            wsb = cp.tile([P, NG], BF16)
            nc.gpsimd.memset(wsb[:], 0.0)
            wps = ptp.tile([P, NG], F32, name="warm", tag="pt")
            for _ in range(16):
                nc.tensor.matmul(wps[:], wsb[:, 0:P], wsb[:], start=True, stop=True)

            for g in range(G):
                # ---- matmul1: t.T[:, g] = sum_c V.T_c.T @ x.T_c ----
                pt = [
                    ptp.tile([P, NG], F32, name=f"pt{g}_{r}", tag="pt")
                    for r in range(RC)
                ]
                xbase = g * KC * NG
                for c in range(KC):
                    for r in range(RC):
                        nc.tensor.matmul(
                            pt[r][:],
                            vsb[:, c * RANK + r * P : c * RANK + (r + 1) * P],
                            xsb[:, xbase + c * NG : xbase + (c + 1) * NG],
                            start=(c == 0),
                            stop=(c == KC - 1),
                        )
                # evict t to bf16; r0 on ACT, r1 on DVE, split in half so
                # matmul2's first stationary is ready ~250 ns after mm1 ends
                h = NG // 2
                for r in range(RC):
                    base = r * TPC + g * NG
                    if r == 0:
                        nc.scalar.copy(tT[:, base : base + h], pt[r][:, :h])
                        nc.scalar.copy(tT[:, base + h : base + NG], pt[r][:, h:])
                    else:
                        nc.vector.tensor_copy(tT[:, base : base + h], pt[r][:, :h])
                        nc.vector.tensor_copy(tT[:, base + h : base + NG], pt[r][:, h:])

                # ---- matmul2: y[tok, of] = t @ U.T + bias ----
                # ofb-pair outer so early of-blocks (whose U lands first) are
                # consumed across all token tiles before later U is needed.
                # PSUM groups span 2 banks (1024 cols) so each eviction op
                # amortizes the ~450 ns fixed engine overhead; evictions are
                # plain converting copies alternating DVE/ACT (bias is added
                # on the host), keeping both engines at half the PE's
                # 854 ns/group pace.
                ysb = [yp.tile([P, OUT_F], BF16, name=f"y{g}_{t}", tag="y") for t in range(TT)]
                NG2 = 2 * NG
                for ofp in range(OFB // 2):
                    for t in range(TT):
                        py = pyp.tile([P, NG2], F32, tag="py")
                        for h in range(2):
                            ofb = 2 * ofp + h
                            for r in range(RC):
                                nc.tensor.matmul(
                                    py[:, h * NG : (h + 1) * NG],
                                    tT[:, r * TPC + g * NG + t * P : r * TPC + g * NG + (t + 1) * P],
                                    usb[:, r * OUT_F + ofb * NG : r * OUT_F + (ofb + 1) * NG],
                                    start=(r == 0),
                                    stop=(r == RC - 1),
                                )
                        ys = ysb[t][:, ofp * NG2 : (ofp + 1) * NG2]
                        if (ofp * TT + t) % 2 == 0:
                            nc.vector.tensor_copy(ys, py[:])
                        else:
                            nc.scalar.copy(ys, py[:])
                    # stores fire per (ofp, t) right behind the evictions so
                    # the outflow spreads across matmul2 instead of bursting
                    # at the end; 2 KB contiguous lines per partition.
                    for t in range(TT):
                        nc.sync.dma_start(
                            yD[
                                g * NG + t * P : g * NG + (t + 1) * P,
                                ofp * NG2 : (ofp + 1) * NG2,
                            ],
                            ysb[t][:, ofp * NG2 : (ofp + 1) * NG2],
                        )
    nc.compile()
    return nc


def _get_nc():
    if "nc" not in _CACHE:
        _CACHE["nc"] = _build()
    return _CACHE["nc"]


def _prep_in_maps(x, U, V, bias):
    x = np.ascontiguousarray(x, dtype=np.float32)
    V = np.asarray(V, dtype=np.float32)
    U = np.asarray(U, dtype=np.float32)
    # vP[p, c*RANK+m] = V[m, c*128+p]
    vp = np.ascontiguousarray(
        V.reshape(RANK, KC, P).transpose(2, 1, 0).reshape(P, KC * RANK).astype(NPBF16)
    )
    # uP[p, r*OUT_F+o] = U[o, r*128+p]
    up = np.ascontiguousarray(
        U.reshape(OUT_F, RC, P).transpose(2, 1, 0).reshape(P, RC * OUT_F).astype(NPBF16)
    )
    in_maps = []
    for i in range(N_CORES):
        xs = x[i * TPC : (i + 1) * TPC, :]
        # xP[p, (g*KC+c)*NG + n] = x[g*NG+n, c*128+p]
        xp_img = np.ascontiguousarray(
            xs.reshape(G, NG, KC, P).transpose(3, 0, 2, 1).reshape(P, G * KC * NG).astype(NPBF16)
        )
        in_maps.append({"xP": xp_img, "vP": vp, "uP": up})
    return in_maps


def _gather(res, bias):
    # res.results[i]["yD"] is [TPC, OUT_F] bf16 in natural token order;
    # bias is added here in f32 (device evictions are plain copies).
    y = np.concatenate([res.results[i]["yD"] for i in range(N_CORES)], axis=0).astype(
        np.float32
    )
    y += np.asarray(bias, dtype=np.float32)[None, :]
    return y


def kernel(x, U, V, bias):
    nc = _get_nc()
    in_maps = _prep_in_maps(x, U, V, bias)
    res = run_bass_kernel_spmd(nc, in_maps, core_ids=list(range(N_CORES)))
    return _gather(res, bias)


def run_profiled(x, U, V, bias, **trace_kwargs):
    """Like kernel() but with NTFF tracing; returns (y, BassKernelResults)."""
    nc = _get_nc()
    in_maps = _prep_in_maps(x, U, V, bias)
    res = run_bass_kernel_spmd(
        nc, in_maps, core_ids=list(range(N_CORES)), trace=True, **trace_kwargs
    )
    return _gather(res, bias), res
